# revision 35
# baseline (speedup 1.0000x reference)
"""Trainium2 Bass kernel for the LogRatio loss (nn_LogRatio_14104672600201).

Math: the reference loss factorizes (see the derivation in the epilogue
below). Every masked reduction over logsim[j, l] = log((X @ X.T)[j, l]) has a
mask depending on j only through targets[j] in [0, 64), so each row-reduction
becomes a GEMM against a label-derived matrix Q[l, g] followed by a per-row
one-hot select at g = targets[j].

Layout (g-major): per core (512-row j-shard, rotated so its own rows sit at
columns 0:512 of the l axis):

  for each of 32 l-tiles:
    sim   = xT_tile.T @ xT[:, 0:512]            # [128 l, 512 j]  PSUM
    y     = Ln(KSC * sim)                       # shifted log, bf16
    y2    = y * y                               # DVE bf16 2x
    X1a  += Qa.T @ y      (Qa = [P | W0])       # [128 g, 512 j]  accum
    X2a  += Qa.T @ y2                           # [128 g, 512 j]  accum
    X1b  += Qb.T @ y      (Qb = [W1])           # [ 64 g, 512 j]  accum

Q is STATIONARY and the full j-shard is the MOVING operand (N=512), so the
PE runs ~134 big matmuls instead of 320 small ones, and LDWEIGHTS drops
3x. All matmul operands are bf16 (the shift trick keeps y in [-0.3, 0.35],
so bf16's 8-bit mantissa costs only ~1e-4 absolute per element).

Selection: the 5 per-j values (yP, yW0, yW1, y2P, y2W0) are extracted
without transposes: multiply the [g, j] accumulators elementwise by the
one-hot mask M[g, j] = (g % 64 == t_j), then contract over partitions with a
tiny 2-column indicator matmul -> [2, 512] rows. Those 6 rows DMA out and
the final scalar loss is reconstructed on host in float64.
"""

import numpy as np
import ml_dtypes

N, D, KK, C = 4096, 128, 4, 64
NCORES = 8
JSH = N // NCORES          # 512 j rows per core
LT = N // 128              # 32 l-tiles
GW = 192                   # Q width: [P(64) | W0(64) | W1(64)]
EPS = 1e-6
OMEGA = 0.1
KSC = float(np.float32(np.exp(-3.5)))        # Ln input scale (exactly f32)
SHIFT = float(-np.log(np.float64(KSC)))      # effective shift s = -ln(KSC)

_CACHE = {}


def _build_nc():
    import concourse.bass as bass
    import concourse.bacc as bacc
    import concourse.mybir as mybir
    import concourse.tile as tile
    from contextlib import ExitStack

    f32 = mybir.dt.float32
    f32r = mybir.dt.float32r
    bf16 = mybir.dt.bfloat16
    Ln = mybir.ActivationFunctionType.Ln

    nc = bacc.Bacc("TRN2", target_bir_lowering=False, debug=False)
    xt = nc.dram_tensor("xt", [D, N], bf16, kind="ExternalInput")
    # q[p, lt*GW + g] = Q[lt*128 + p, g]
    q = nc.dram_tensor("q", [128, LT * GW], bf16, kind="ExternalInput")
    # mask[g, j] = (g % 64 == t_j), stacked twice along g
    msk = nc.dram_tensor("msk", [128, JSH], bf16, kind="ExternalInput")
    # indicator: ind[g] = [(g < 64), (g >= 64), 1.0]
    ind = nc.dram_tensor("ind", [128, 3], bf16, kind="ExternalInput")
    # out rows: [yP, yW0 | yW1, 0 | y2P, y2W0]
    lout = nc.dram_tensor("lout", [2, 3 * JSH], f32, kind="ExternalOutput")

    NSIM = 4   # explicit sim PSUM tiles (deterministic 4-group WAW slack)
    NLS = 8    # explicit ls/ls2 tiles (Ln never waits on recent readers)

    with tile.TileContext(nc) as tc, ExitStack() as ctx:
        cpool = ctx.enter_context(tc.tile_pool(name="const", bufs=1))
        mpool = ctx.enter_context(tc.tile_pool(name="mpool", bufs=1))
        px = ctx.enter_context(tc.tile_pool(name="px", bufs=1, space="PSUM"))

        # ---- PE warm-up: a memset scratch tile (no DMA dependency) feeds
        # back-to-back dummy matmuls from ~1.5us, so the HAM clock-gate opens
        # (1.2 -> 2.4 GHz) before real data lands and the real stream starts
        # warm. Output bank is never read.
        scr = cpool.tile([128, JSH], bf16, tag="scr")
        nc.vector.memset(scr[:], 0.0)
        warm = px.tile([128, JSH], f32, tag="warm", name="warm")
        for _ in range(2):
            nc.tensor.matmul(
                warm[:], scr[:, 0:128], scr[:], start=True, stop=True
            )

        # ---- DMAs: first chunks feed the first sim matmuls; issue from two
        # engine queues (each DMA_DIRECT2D costs ~0.6us serial on its queue).
        xt_sb = cpool.tile([D, N], bf16, tag="xt")
        nc.sync.dma_start(xt_sb[:, 0:512], xt[:, 0:512])
        q_sb = cpool.tile([128, LT * GW], bf16, tag="q")
        nc.sync.dma_start(q_sb[:, 0:768], q[:, 0:768])
        nc.sync.dma_start(xt_sb[:, 512:2304], xt[:, 512:2304])
        nc.sync.dma_start(q_sb[:, 768:3456], q[:, 768:3456])
        nc.sync.dma_start(xt_sb[:, 2304:4096], xt[:, 2304:4096])
        nc.sync.dma_start(q_sb[:, 3456:6144], q[:, 3456:6144])
        msk_sb = cpool.tile([128, JSH], bf16, tag="msk")
        nc.sync.dma_start(msk_sb[:], msk[:])
        ind_sb = cpool.tile([128, 3], bf16, tag="ind")
        nc.sync.dma_start(ind_sb[:], ind[:])

        # accumulators (one PSUM bank each, held across the whole lt loop).
        # x1b holds even-lt W1 sums on partitions 0:64 and odd-lt sums on
        # 64:128 (column-tiled pairs); the all-ones indicator column sums the
        # halves inside the collapse matmul.
        x1a = px.tile([128, JSH], f32, tag="x1a")
        x2a = px.tile([128, JSH], f32, tag="x2a")
        x1b = px.tile([128, JSH], f32, tag="x1b")

        # explicit round-robin tiles: reuse edges are fixed WAW/WAR deps with
        # NSIM/NLS groups of slack (a pooled allocator may pick the most
        # recently freed slot, creating a 1-group PE<->Scalar cycle).
        simps = [
            px.tile([128, JSH], f32, tag=f"simp{i}", name=f"simp{i}")
            for i in range(NSIM)
        ]
        lss = [
            cpool.tile([128, JSH], bf16, tag=f"ls{i}", name=f"ls{i}")
            for i in range(NLS)
        ]
        ls2s = [
            cpool.tile([128, JSH], bf16, tag=f"ls2{i}", name=f"ls2{i}")
            for i in range(NLS)
        ]

        mov = xt_sb[:, 0:JSH]

        def sim_stage(lt):
            simp = simps[lt % NSIM]
            nc.tensor.matmul(
                simp[:], xt_sb[:, bass.ts(lt, 128)], mov, start=True, stop=True
            )
            ls = lss[lt % NLS]
            nc.scalar.activation(ls[:], simp[:], Ln, scale=KSC)
            ls2 = ls2s[lt % NLS]
            nc.vector.tensor_mul(ls2[:], ls[:], ls[:])

        def qslice(lt, a, b):
            return q_sb[:, lt * GW + a : lt * GW + b]

        def x1b_pair(le, sp):
            # two 64-wide W1 matmuls on distinct column strips run
            # concurrently on the PE array; deferred one group so the qb
            # weight loads have a full group of prefetch lead
            nc.tensor.matmul(
                x1b[0:64, :], qslice(le, 128, GW), lss[le % NLS][:],
                start=le == 0, stop=sp, tile_position=(0, 0),
            )
            nc.tensor.matmul(
                x1b[64:128, :], qslice(le + 1, 128, GW),
                lss[(le + 1) % NLS][:],
                start=le == 0, stop=sp, tile_position=(0, 64),
            )

        for lt in range(NSIM):
            sim_stage(lt)
        for lt in range(LT):
            if lt + NSIM < LT:
                sim_stage(lt + NSIM)
            if lt in (0, 1, 2, 3, 5, 7):
                # dependency-free keepalive: fills any early DMA-stall idle
                # so the HAM activity window never re-throttles the PE clock
                nc.tensor.matmul(
                    warm[:], scr[:, 0:128], scr[:], start=True, stop=True
                )
            if lt % 2 == 0 and lt >= 2:
                x1b_pair(lt - 2, False)
            if lt == LT - 1:
                # final pair first, so x1b's tail chain starts earliest
                x1b_pair(LT - 2, True)
            qa = qslice(lt, 0, 128)
            st = lt == 0
            sp = lt == LT - 1
            nc.tensor.matmul(x1a[:], qa, lss[lt % NLS][:], start=st, stop=sp)
            nc.tensor.matmul(x2a[:], qa, ls2s[lt % NLS][:], start=st, stop=sp)

        # ---- selection: mask-mul then indicator collapse matmul. The sel
        # PSUM tiles reuse the (now dead) accumulator banks via pool tags.
        sel_sb = mpool.tile([2, 3 * JSH], f32, tag="selsb")
        srcs = (x1a, x1b, x2a)
        ms = [None] * 3
        # emission order matches accumulator stop order: x1b first, x2a last
        for i in (1, 0, 2):
            m = mpool.tile([128, JSH], bf16, tag=f"m{i}", name=f"m{i}")
            nc.vector.tensor_mul(m[:], srcs[i][:], msk_sb[:])
            ms[i] = m
        for i in (1, 0, 2):
            # x1a/x2a: [P-sel | W0-sel] rows; x1b: ones column sums both
            # partition halves -> W1-sel on row 0
            isl = ind_sb[:, 2:3] if i == 1 else ind_sb[:, 0:2]
            nr = 1 if i == 1 else 2
            sel = px.tile([2, JSH], f32, tag=("x1a", "x1b", "x2a")[i],
                          name=f"sel{i}")
            nc.tensor.matmul(
                sel[0:nr, :], isl, ms[i][:], start=True, stop=True
            )
            # copy on the (idle at tail) scalar engine; DVE runs mask-muls
            nc.scalar.activation(
                sel_sb[0:nr, bass.ts(i, JSH)], sel[0:nr, :],
                mybir.ActivationFunctionType.Copy,
            )
            # fire each block's out-DMA as soon as its copy lands
            nc.sync.dma_start(
                lout[0:nr, bass.ts(i, JSH)], sel_sb[0:nr, bass.ts(i, JSH)]
            )
    nc.compile()
    return nc


def _host_prep(inputs, labels):
    x = np.asarray(inputs, dtype=np.float32)
    lab = np.asarray(labels)
    t = lab[:, 0].astype(np.int64)
    bf = ml_dtypes.bfloat16

    m = np.arange(KK)
    om = np.float64(OMEGA)
    lp = np.log(np.float64(OMEGA + EPS)) - np.log(om ** (KK - m + 1) + np.float64(EPS))

    gr = np.arange(C)
    eq = lab[None, :, :] == gr[:, None, None]          # [C, N, KK]
    nm = np.stack(
        [
            ~eq[:, :, 3],
            eq[:, :, 3] & ~eq[:, :, 2],
            eq[:, :, 2] & ~eq[:, :, 1],
            eq[:, :, 1] & ~eq[:, :, 0],
        ]
    ).astype(np.float64)                                # [KK, C, N]
    w0 = nm.sum(0)                                      # [C, N]
    w1 = np.einsum("m,mcl->cl", lp, nm)
    w2 = np.einsum("m,mcl->cl", lp * lp, nm)
    ph = (t[:, None] == gr[None, :]).astype(np.float64)  # [N, C] one-hot t_l

    qm = np.zeros((N, GW), dtype=np.float32)
    qm[:, 0:C] = ph
    qm[:, C : 2 * C] = w0.T
    qm[:, 2 * C : 3 * C] = w1.T

    ind = np.zeros((128, 3), dtype=np.float32)
    ind[0:64, 0] = 1.0
    ind[64:128, 1] = 1.0
    ind[:, 2] = 1.0

    xt = np.ascontiguousarray(x.T)                       # [D, N]
    in_maps = []
    for cid in range(NCORES):
        sl = slice(cid * JSH, (cid + 1) * JSH)
        # rotate the l axis so this core's own j-shard sits at columns
        # 0:JSH; the l reduction (over all 4096) is rotation-invariant as
        # long as q's rows rotate identically.
        xtc = np.roll(xt, -cid * JSH, axis=1)
        qc = np.roll(qm, -cid * JSH, axis=0)             # [N, GW]
        # q_sb[p, lt*GW + g] = Q[lt*128 + p, g]
        qsb = np.ascontiguousarray(
            qc.reshape(LT, 128, GW).transpose(1, 0, 2).reshape(128, LT * GW)
        )
        oh = (gr[:, None] == t[sl][None, :]).astype(np.float32)  # [64, 512]
        mk = np.concatenate([oh, oh], axis=0)            # [128, 512]
        in_maps.append(
            {
                "xt": xtc.astype(bf),
                "q": qsb.astype(bf),
                "msk": mk.astype(bf),
                "ind": ind.astype(bf),
            }
        )

    tabs = {
        "t": t, "cnt": ph.sum(0), "h0": w0.sum(1), "h1": w1.sum(1),
        "h2": w2.sum(1), "x": x,
    }
    return in_maps, tabs


def _host_loss(res_list, tabs):
    t, cnt, h0, h1, h2 = tabs["t"], tabs["cnt"], tabs["h0"], tabs["h1"], tabs["h2"]
    x64 = tabs["x"].astype(np.float64)
    s = np.float64(SHIFT)
    loss = np.float64(0.0)
    for cid, r in enumerate(res_list):
        sl = slice(cid * JSH, (cid + 1) * JSH)
        lo = r["lout"].astype(np.float64)                # [2, 3*JSH]
        yP, yW0 = lo[0, 0:JSH], lo[1, 0:JSH]
        yW1 = lo[0, JSH : 2 * JSH]
        y2P, y2W0 = lo[0, 2 * JSH :], lo[1, 2 * JSH :]
        tj = t[sl]
        cj, h0j, h1j, h2j = cnt[tj], h0[tj], h1[tj], h2[tj]
        diag = np.log(np.einsum("jd,jd->j", x64[sl], x64[sl]) + EPS)
        S1 = yP + s * cj - diag
        S2 = y2P + 2 * s * yP + s * s * cj - diag * diag
        A1 = yW0 + s * h0j + 0.1 * h1j
        A2 = (y2W0 + 2 * s * yW0 + s * s * h0j) + 0.2 * (yW1 + s * h1j) + 0.01 * h2j
        loss += np.sum(S2 * h0j - 2.0 * S1 * A1 + (cj - 1.0) * A2)
    return np.array(loss, dtype=np.float32)


def _run(inputs, labels, trace=False, tmpdir=None):
    from concourse.bass_utils import run_bass_kernel_spmd

    if "nc" not in _CACHE:
        _CACHE["nc"] = _build_nc()
    in_maps, tabs = _host_prep(inputs, labels)
    res = run_bass_kernel_spmd(
        _CACHE["nc"], in_maps, core_ids=list(range(NCORES)),
        trace=trace, tmpdir=tmpdir,
    )
    return _host_loss(res.results, tabs), res


def kernel(inputs, labels):
    out, _ = _run(inputs, labels, trace=False)
    return out


# revision 41
# speedup vs baseline: 1.0256x; 1.0256x over previous
"""Trainium2 Bass kernel for the LogRatio loss (nn_LogRatio_14104672600201).

Math: the reference loss factorizes (see the derivation in the epilogue
below). Every masked reduction over logsim[j, l] = log((X @ X.T)[j, l]) has a
mask depending on j only through targets[j] in [0, 64), so each row-reduction
becomes a GEMM against a label-derived matrix Q[l, g] followed by a per-row
one-hot select at g = targets[j].

Layout (g-major): per core (512-row j-shard, rotated so its own rows sit at
columns 0:512 of the l axis):

  for each of 32 l-tiles:
    sim   = xT_tile.T @ xT[:, 0:512]            # [128 l, 512 j]  PSUM
    y     = Ln(KSC * sim)                       # shifted log, bf16
    y2    = y * y                               # DVE bf16 2x
    X1a  += Qa.T @ y      (Qa = [P | W0])       # [128 g, 512 j]  accum
    X2a  += Qa.T @ y2                           # [128 g, 512 j]  accum
    X1b  += Qb.T @ y      (Qb = [W1])           # [ 64 g, 512 j]  accum

Q is STATIONARY and the full j-shard is the MOVING operand (N=512), so the
PE runs ~134 big matmuls instead of 320 small ones, and LDWEIGHTS drops
3x. All matmul operands are bf16 (the shift trick keeps y in [-0.3, 0.35],
so bf16's 8-bit mantissa costs only ~1e-4 absolute per element).

Selection: the 5 per-j values (yP, yW0, yW1, y2P, y2W0) are extracted
without transposes: multiply the [g, j] accumulators elementwise by the
one-hot mask M[g, j] = (g % 64 == t_j), then contract over partitions with a
tiny 2-column indicator matmul -> [2, 512] rows. Those 6 rows DMA out and
the final scalar loss is reconstructed on host in float64.
"""

import numpy as np
import ml_dtypes

N, D, KK, C = 4096, 128, 4, 64
NCORES = 8
JSH = N // NCORES          # 512 j rows per core
LT = N // 128              # 32 l-tiles
GW = 192                   # Q width: [P(64) | W0(64) | W1(64)]
EPS = 1e-6
OMEGA = 0.1
KSC = float(np.float32(np.exp(-3.5)))        # Ln input scale (exactly f32)
SHIFT = float(-np.log(np.float64(KSC)))      # effective shift s = -ln(KSC)

_CACHE = {}


def _build_nc():
    import concourse.bass as bass
    import concourse.bacc as bacc
    import concourse.mybir as mybir
    import concourse.tile as tile
    from contextlib import ExitStack

    f32 = mybir.dt.float32
    f32r = mybir.dt.float32r
    bf16 = mybir.dt.bfloat16
    Ln = mybir.ActivationFunctionType.Ln

    nc = bacc.Bacc("TRN2", target_bir_lowering=False, debug=False)
    xt = nc.dram_tensor("xt", [D, N], bf16, kind="ExternalInput")
    # q[p, lt*GW + g] = Q[lt*128 + p, g]
    q = nc.dram_tensor("q", [128, LT * GW], bf16, kind="ExternalInput")
    # mask[g, j] = (g % 64 == t_j), stacked twice along g
    msk = nc.dram_tensor("msk", [128, JSH], bf16, kind="ExternalInput")
    # masked accumulators [x1a | x1b | x2a]; host does the partition sums
    lout = nc.dram_tensor("lout", [128, 3 * JSH], bf16, kind="ExternalOutput")

    NSIM = 4   # explicit sim PSUM tiles (deterministic 4-group WAW slack)
    NLS = 8    # explicit ls/ls2 tiles (Ln never waits on recent readers)

    with tile.TileContext(nc) as tc, ExitStack() as ctx:
        cpool = ctx.enter_context(tc.tile_pool(name="const", bufs=1))
        mpool = ctx.enter_context(tc.tile_pool(name="mpool", bufs=1))
        px = ctx.enter_context(tc.tile_pool(name="px", bufs=1, space="PSUM"))

        # ---- PE warm-up: a memset scratch tile (no DMA dependency) feeds
        # back-to-back dummy matmuls from ~1.5us, so the HAM clock-gate opens
        # (1.2 -> 2.4 GHz) before real data lands and the real stream starts
        # warm. Output bank is never read.
        scr = cpool.tile([128, JSH], bf16, tag="scr")
        nc.vector.memset(scr[:], 0.0)
        warm = px.tile([128, JSH], f32, tag="warm", name="warm")
        for _ in range(2):
            nc.tensor.matmul(
                warm[:], scr[:, 0:128], scr[:], start=True, stop=True
            )

        # ---- DMAs: first chunks feed the first sim matmuls; issue from two
        # engine queues (each DMA_DIRECT2D costs ~0.6us serial on its queue).
        xt_sb = cpool.tile([D, N], bf16, tag="xt")
        nc.sync.dma_start(xt_sb[:, 0:512], xt[:, 0:512])
        q_sb = cpool.tile([128, LT * GW], bf16, tag="q")
        nc.sync.dma_start(q_sb[:, 0:768], q[:, 0:768])
        nc.sync.dma_start(xt_sb[:, 512:2304], xt[:, 512:2304])
        nc.sync.dma_start(q_sb[:, 768:3456], q[:, 768:3456])
        nc.sync.dma_start(xt_sb[:, 2304:4096], xt[:, 2304:4096])
        nc.sync.dma_start(q_sb[:, 3456:6144], q[:, 3456:6144])
        msk_sb = cpool.tile([128, JSH], bf16, tag="msk")
        nc.sync.dma_start(msk_sb[:], msk[:])

        # accumulators (one PSUM bank each, held across the whole lt loop).
        # x1b holds even-lt W1 sums on partitions 0:64 and odd-lt sums on
        # 64:128 (column-tiled pairs); the all-ones indicator column sums the
        # halves inside the collapse matmul.
        x1a = px.tile([128, JSH], f32, tag="x1a")
        x2a = px.tile([128, JSH], f32, tag="x2a")
        x1b = px.tile([128, JSH], f32, tag="x1b")

        # explicit round-robin tiles: reuse edges are fixed WAW/WAR deps with
        # NSIM/NLS groups of slack (a pooled allocator may pick the most
        # recently freed slot, creating a 1-group PE<->Scalar cycle).
        simps = [
            px.tile([128, JSH], f32, tag=f"simp{i}", name=f"simp{i}")
            for i in range(NSIM)
        ]
        lss = [
            cpool.tile([128, JSH], bf16, tag=f"ls{i}", name=f"ls{i}")
            for i in range(NLS)
        ]
        ls2s = [
            cpool.tile([128, JSH], bf16, tag=f"ls2{i}", name=f"ls2{i}")
            for i in range(NLS)
        ]

        mov = xt_sb[:, 0:JSH]

        def sim_stage(lt):
            simp = simps[lt % NSIM]
            nc.tensor.matmul(
                simp[:], xt_sb[:, bass.ts(lt, 128)], mov, start=True, stop=True
            )
            ls = lss[lt % NLS]
            nc.scalar.activation(ls[:], simp[:], Ln, scale=KSC)
            ls2 = ls2s[lt % NLS]
            nc.vector.tensor_mul(ls2[:], ls[:], ls[:])

        def qslice(lt, a, b):
            return q_sb[:, lt * GW + a : lt * GW + b]

        def x1b_pair(le, sp):
            # two 64-wide W1 matmuls on distinct column strips run
            # concurrently on the PE array; deferred one group so the qb
            # weight loads have a full group of prefetch lead
            nc.tensor.matmul(
                x1b[0:64, :], qslice(le, 128, GW), lss[le % NLS][:],
                start=le == 0, stop=sp, tile_position=(0, 0),
            )
            nc.tensor.matmul(
                x1b[64:128, :], qslice(le + 1, 128, GW),
                lss[(le + 1) % NLS][:],
                start=le == 0, stop=sp, tile_position=(0, 64),
            )

        for lt in range(NSIM):
            sim_stage(lt)
        for lt in range(LT):
            if lt + NSIM < LT:
                sim_stage(lt + NSIM)
            if lt in (0, 1, 2, 3, 5, 7):
                # dependency-free keepalive: fills any early DMA-stall idle
                # so the HAM activity window never re-throttles the PE clock
                nc.tensor.matmul(
                    warm[:], scr[:, 0:128], scr[:], start=True, stop=True
                )
            if lt % 2 == 0 and lt >= 2:
                x1b_pair(lt - 2, False)
            if lt == LT - 1:
                # final pair first, so x1b's tail chain starts earliest
                x1b_pair(LT - 2, True)
            qa = qslice(lt, 0, 128)
            st = lt == 0
            sp = lt == LT - 1
            nc.tensor.matmul(x1a[:], qa, lss[lt % NLS][:], start=st, stop=sp)
            nc.tensor.matmul(x2a[:], qa, ls2s[lt % NLS][:], start=st, stop=sp)

        # ---- selection: mask-mul, then DMA the masked tiles straight out;
        # the host does the (tiny) partition sums in float64.
        srcs = (x1a, x1b, x2a)
        # emission order matches accumulator stop order: x1b first, x2a last
        for i in (1, 0, 2):
            m = mpool.tile([128, JSH], bf16, tag=f"m{i}", name=f"m{i}")
            nc.vector.tensor_mul(m[:], srcs[i][:], msk_sb[:])
            nc.sync.dma_start(lout[:, bass.ts(i, JSH)], m[:])
    nc.compile()
    return nc


def _host_prep(inputs, labels):
    x = np.asarray(inputs, dtype=np.float32)
    lab = np.asarray(labels)
    t = lab[:, 0].astype(np.int64)
    bf = ml_dtypes.bfloat16

    m = np.arange(KK)
    om = np.float64(OMEGA)
    lp = np.log(np.float64(OMEGA + EPS)) - np.log(om ** (KK - m + 1) + np.float64(EPS))

    gr = np.arange(C)
    eq = lab[None, :, :] == gr[:, None, None]          # [C, N, KK]
    nm = np.stack(
        [
            ~eq[:, :, 3],
            eq[:, :, 3] & ~eq[:, :, 2],
            eq[:, :, 2] & ~eq[:, :, 1],
            eq[:, :, 1] & ~eq[:, :, 0],
        ]
    ).astype(np.float64)                                # [KK, C, N]
    w0 = nm.sum(0)                                      # [C, N]
    w1 = np.einsum("m,mcl->cl", lp, nm)
    w2 = np.einsum("m,mcl->cl", lp * lp, nm)
    ph = (t[:, None] == gr[None, :]).astype(np.float64)  # [N, C] one-hot t_l

    qm = np.zeros((N, GW), dtype=np.float32)
    qm[:, 0:C] = ph
    qm[:, C : 2 * C] = w0.T
    qm[:, 2 * C : 3 * C] = w1.T

    xt = np.ascontiguousarray(x.T)                       # [D, N]
    in_maps = []
    for cid in range(NCORES):
        sl = slice(cid * JSH, (cid + 1) * JSH)
        # rotate the l axis so this core's own j-shard sits at columns
        # 0:JSH; the l reduction (over all 4096) is rotation-invariant as
        # long as q's rows rotate identically.
        xtc = np.roll(xt, -cid * JSH, axis=1)
        qc = np.roll(qm, -cid * JSH, axis=0)             # [N, GW]
        # q_sb[p, lt*GW + g] = Q[lt*128 + p, g]
        qsb = np.ascontiguousarray(
            qc.reshape(LT, 128, GW).transpose(1, 0, 2).reshape(128, LT * GW)
        )
        oh = (gr[:, None] == t[sl][None, :]).astype(np.float32)  # [64, 512]
        mk = np.concatenate([oh, oh], axis=0)            # [128, 512]
        in_maps.append(
            {
                "xt": xtc.astype(bf),
                "q": qsb.astype(bf),
                "msk": mk.astype(bf),
            }
        )

    tabs = {
        "t": t, "cnt": ph.sum(0), "h0": w0.sum(1), "h1": w1.sum(1),
        "h2": w2.sum(1), "x": x,
    }
    return in_maps, tabs


def _host_loss(res_list, tabs):
    t, cnt, h0, h1, h2 = tabs["t"], tabs["cnt"], tabs["h0"], tabs["h1"], tabs["h2"]
    x64 = tabs["x"].astype(np.float64)
    s = np.float64(SHIFT)
    loss = np.float64(0.0)
    for cid, r in enumerate(res_list):
        sl = slice(cid * JSH, (cid + 1) * JSH)
        lo = r["lout"].astype(np.float64)                # [128, 3*JSH]
        m0, m1, m2 = lo[:, 0:JSH], lo[:, JSH : 2 * JSH], lo[:, 2 * JSH :]
        yP, yW0 = m0[0:64].sum(0), m0[64:128].sum(0)
        yW1 = m1.sum(0)
        y2P, y2W0 = m2[0:64].sum(0), m2[64:128].sum(0)
        tj = t[sl]
        cj, h0j, h1j, h2j = cnt[tj], h0[tj], h1[tj], h2[tj]
        diag = np.log(np.einsum("jd,jd->j", x64[sl], x64[sl]) + EPS)
        S1 = yP + s * cj - diag
        S2 = y2P + 2 * s * yP + s * s * cj - diag * diag
        A1 = yW0 + s * h0j + 0.1 * h1j
        A2 = (y2W0 + 2 * s * yW0 + s * s * h0j) + 0.2 * (yW1 + s * h1j) + 0.01 * h2j
        loss += np.sum(S2 * h0j - 2.0 * S1 * A1 + (cj - 1.0) * A2)
    return np.array(loss, dtype=np.float32)


def _run(inputs, labels, trace=False, tmpdir=None):
    from concourse.bass_utils import run_bass_kernel_spmd

    if "nc" not in _CACHE:
        _CACHE["nc"] = _build_nc()
    in_maps, tabs = _host_prep(inputs, labels)
    res = run_bass_kernel_spmd(
        _CACHE["nc"], in_maps, core_ids=list(range(NCORES)),
        trace=trace, tmpdir=tmpdir,
    )
    return _host_loss(res.results, tabs), res


def kernel(inputs, labels):
    out, _ = _run(inputs, labels, trace=False)
    return out
